# revision 2
# baseline (speedup 1.0000x reference)
"""Trainium2 Bass kernel for nn_Decompressor (LSTM decompressor).

Reference computation (see problem statement):
    T=256 steps of an LSTM (batch B=128, hidden P=1024) whose output feeds
    back as its input, followed by a linear projection to E=1024:
        gates_t = xin @ W_ih.T + h @ W_hh.T + (b_ih + b_hh)
        i,f,g,o = split(gates_t); c = sig(f)*c + sig(i)*tanh(g)
        h = sig(o)*tanh(c);  xin_{t+1} = h
        out[:, t, :] = h_t @ W_out.T + b_out
    Since xin == h for t>=1, gates_t = h @ (W_ih + W_hh).T + b for t>=1.

Distribution (8 NeuronCores, SPMD):
    Model-parallel over the hidden dimension: core j owns hidden units
    [128j, 128j+128). Each step, core j computes its 512 gate columns
    (i|f|o|g of its units) as a full-batch matmul (M=B=128, K=1024, N=512,
    float32r), applies the LSTM cell pointwise, transposes its h-slice, and
    an AllGather reassembles the full h^T (the next step's stationary
    operand) on every core. The output projection is sharded over E: core j
    computes out^T[Ej, :, :] from the gathered h^T, batching 2 timesteps per
    matmul (N=256) so float32r runs at full rate.

Host side: weight reordering/slicing, x transpose, and final concat/
transpose of the per-core [E_j, T, B] outputs back to [B, T, E].
"""

import numpy as np

import concourse.bacc as bacc
import concourse.mybir as mybir
import concourse.tile as tile
from concourse.bass_utils import run_bass_kernel_spmd

B = 128
P = 1024
E = 1024
T = 256
NC = 8
KT = P // 128          # 8 K-tiles
GS = 4 * P // NC       # 512 gate columns per core
ES = E // NC           # 128 output columns per core

F32 = mybir.dt.float32
F32R = mybir.dt.float32r


def _build(t_steps=T):
    nc = bacc.Bacc("TRN2", target_bir_lowering=False, debug=False, num_devices=NC)

    wrec_d = nc.dram_tensor("wrec", [128, KT * GS], F32R, kind="ExternalInput").ap()
    win_d = nc.dram_tensor("win", [128, KT * GS], F32R, kind="ExternalInput").ap()
    xT_d = nc.dram_tensor("xT", [128, KT * B], F32R, kind="ExternalInput").ap()
    bias_d = nc.dram_tensor("bias", [1, GS], F32R, kind="ExternalInput").ap()
    wout_d = nc.dram_tensor("wout", [128, KT * ES], F32R, kind="ExternalInput").ap()
    bout_d = nc.dram_tensor("bout", [128, 1], F32, kind="ExternalInput").ap()
    ones_d = nc.dram_tensor("ones", [1, 128], F32R, kind="ExternalInput").ap()
    ident_d = nc.dram_tensor("ident", [128, 128], F32R, kind="ExternalInput").ap()
    outT_d = nc.dram_tensor("outT", [ES, t_steps, B], F32, kind="ExternalOutput").ap()

    with tile.TileContext(nc) as tc:
        with (
            tc.tile_pool(name="const", bufs=1) as cpool,
            tc.tile_pool(name="state", bufs=1) as spool,
            tc.tile_pool(name="work", bufs=2) as wk,
            tc.tile_pool(name="pair", bufs=2) as pairp,
            tc.tile_pool(name="psg", bufs=2, space="PSUM") as psg,
            tc.tile_pool(name="pst", bufs=2, space="PSUM") as pst,
            tc.tile_pool(name="psp", bufs=2, space="PSUM") as psp,
            tc.tile_pool(name="dram", bufs=3, space="DRAM") as dram,
        ):
            wrec_sb = cpool.tile([128, KT * GS], F32R)
            win_sb = cpool.tile([128, KT * GS], F32R)
            xT_sb = cpool.tile([128, KT * B], F32R)
            bias_sb = cpool.tile([1, GS], F32R)
            wout_sb = cpool.tile([128, KT * ES], F32R)
            bout_sb = cpool.tile([128, 1], F32)
            ones_sb = cpool.tile([1, 128], F32R)
            ident_sb = cpool.tile([128, 128], F32R)
            nc.sync.dma_start(wrec_sb[:], wrec_d[:])
            nc.sync.dma_start(win_sb[:], win_d[:])
            nc.sync.dma_start(xT_sb[:], xT_d[:])
            nc.sync.dma_start(bias_sb[:], bias_d[:])
            nc.sync.dma_start(wout_sb[:], wout_d[:])
            nc.sync.dma_start(bout_sb[:], bout_d[:])
            nc.sync.dma_start(ones_sb[:], ones_d[:])
            nc.sync.dma_start(ident_sb[:], ident_d[:])

            c_sb = spool.tile([128, 128], F32)

            pair = None
            for s in range(t_steps):
                # ---- gates matmul: [B, GS] psum, bias via K=1 matmul ----
                ps_g = psg.tile([128, GS], F32, tag="ps_g")
                nc.tensor.matmul(ps_g[:], ones_sb[:], bias_sb[:],
                                 start=True, stop=False)
                w_sb = win_sb if s == 0 else wrec_sb
                for k in range(KT):
                    if s == 0:
                        lhsT = xT_sb[:, k * B:(k + 1) * B]
                    else:
                        slot = (s - 1) % 2
                        lhsT = pair[:, slot * P + k * 128: slot * P + (k + 1) * 128]
                    nc.tensor.matmul(ps_g[:], lhsT, w_sb[:, k * GS:(k + 1) * GS],
                                     start=False, stop=(k == KT - 1))

                # ---- pointwise: layout [i|f|o|g] each 128 wide ----
                gates_sb = wk.tile([128, GS], F32, tag="gates")
                nc.scalar.activation(gates_sb[:, 0:384], ps_g[:, 0:384],
                                     mybir.ActivationFunctionType.Sigmoid)
                nc.scalar.activation(gates_sb[:, 384:512], ps_g[:, 384:512],
                                     mybir.ActivationFunctionType.Tanh)
                i_ap = gates_sb[:, 0:128]
                f_ap = gates_sb[:, 128:256]
                o_ap = gates_sb[:, 256:384]
                g_ap = gates_sb[:, 384:512]
                if s == 0:
                    nc.vector.tensor_tensor(c_sb[:], i_ap, g_ap,
                                            mybir.AluOpType.mult)
                else:
                    ig = wk.tile([128, 128], F32, tag="ig")
                    nc.vector.tensor_tensor(ig[:], i_ap, g_ap,
                                            mybir.AluOpType.mult)
                    nc.vector.tensor_tensor(c_sb[:], c_sb[:], f_ap,
                                            mybir.AluOpType.mult)
                    nc.vector.tensor_tensor(c_sb[:], c_sb[:], ig[:],
                                            mybir.AluOpType.add)
                th = wk.tile([128, 128], F32, tag="th")
                nc.scalar.activation(th[:], c_sb[:],
                                     mybir.ActivationFunctionType.Tanh)
                h_sb = wk.tile([128, 128], F32R, tag="h")
                nc.vector.tensor_tensor(h_sb[:], o_ap, th[:],
                                        mybir.AluOpType.mult)

                # ---- transpose h slice -> [units, B] and exchange ----
                ps_t = pst.tile([128, 128], F32R, tag="ps_t")
                nc.tensor.transpose(ps_t[:], h_sb[:], ident_sb[:])
                hT_send = wk.tile([128, 128], F32R, tag="hTs")
                nc.vector.tensor_copy(hT_send[:], ps_t[:])

                inb = dram.tile([128, B], F32R, tag="inb")
                outb = dram.tile([NC * 128, B], F32R, tag="outb")
                nc.sync.dma_start(inb[:], hT_send[:])
                nc.gpsimd.collective_compute(
                    "AllGather", mybir.AluOpType.bypass,
                    ins=[inb.opt()], outs=[outb.opt()],
                    replica_groups=[list(range(NC))],
                )
                if s % 2 == 0:
                    pair = pairp.tile([128, 2 * P], F32R, tag="pair")
                slot = s % 2
                src3 = outb.opt().rearrange("(k p) b -> p k b", p=128)
                for k in range(KT):
                    nc.sync.dma_start(
                        pair[:, slot * P + k * 128: slot * P + (k + 1) * 128],
                        src3[:, k, :],
                    )

                # ---- projection of steps (s-1, s) every odd step ----
                if s % 2 == 1 or s == t_steps - 1:
                    if s % 2 == 1:
                        nsteps, base = 2, s - 1
                    else:  # odd t_steps tail: project the single last step
                        nsteps, base = 1, s
                    ps_p = psp.tile([128, nsteps * B], F32, tag="ps_p")
                    rhs4 = pair[:].rearrange("p (s2 k b) -> p s2 k b", s2=2, k=KT)
                    for k in range(KT):
                        rhs = rhs4[:, 0:nsteps, k, :]
                        nc.tensor.matmul(ps_p[:], wout_sb[:, k * ES:(k + 1) * ES],
                                         rhs, start=(k == 0), stop=(k == KT - 1))
                    out_sb = wk.tile([128, nsteps * B], F32, tag="out_sb")
                    nc.scalar.activation(out_sb[:], ps_p[:],
                                         mybir.ActivationFunctionType.Identity,
                                         bias=bout_sb[:, 0:1])
                    dst = outT_d[:, base:base + nsteps, :]
                    nc.sync.dma_start(
                        dst, out_sb[:].rearrange("p (s2 b) -> p s2 b", s2=nsteps))

    nc.compile()
    return nc


def _prep_inputs(x, W_ih, W_hh, b_ih, b_hh, W_out, b_out):
    x = np.asarray(x, np.float32)
    W_ih = np.asarray(W_ih, np.float32)
    W_hh = np.asarray(W_hh, np.float32)
    b_ih = np.asarray(b_ih, np.float32)
    b_hh = np.asarray(b_hh, np.float32)
    W_out = np.asarray(W_out, np.float32)
    b_out = np.asarray(b_out, np.float32)

    Wsum = W_ih + W_hh
    bsum = b_ih + b_hh
    # xT as [8 K-tiles, 128, B] laid out [128, KT*B] per tile columns:
    xT_tiles = np.ascontiguousarray(x.T).reshape(KT, 128, B)
    xT_flat = np.concatenate([xT_tiles[k] for k in range(KT)], axis=1)

    ones = np.ones((1, 128), np.float32)
    ident = np.eye(128, dtype=np.float32)

    in_maps = []
    for j in range(NC):
        js = np.arange(128 * j, 128 * (j + 1))
        rows = np.concatenate([0 * P + js, 1 * P + js, 3 * P + js, 2 * P + js])
        Wj = Wsum[rows, :]                       # [512, 1024]
        wrec_t = np.ascontiguousarray(Wj.T).reshape(KT, 128, GS)
        wrec = np.concatenate([wrec_t[k] for k in range(KT)], axis=1)
        Wji = W_ih[rows, :]
        win_t = np.ascontiguousarray(Wji.T).reshape(KT, 128, GS)
        win = np.concatenate([win_t[k] for k in range(KT)], axis=1)
        bias = bsum[rows][None, :]
        Woj = W_out[128 * j:128 * (j + 1), :]    # [128, 1024]
        wout_t = np.ascontiguousarray(Woj.T).reshape(KT, 128, ES)
        wout = np.concatenate([wout_t[k] for k in range(KT)], axis=1)
        bout = b_out[128 * j:128 * (j + 1)][:, None]
        in_maps.append({
            "wrec": np.ascontiguousarray(wrec),
            "win": np.ascontiguousarray(win),
            "xT": np.ascontiguousarray(xT_flat),
            "bias": np.ascontiguousarray(bias),
            "wout": np.ascontiguousarray(wout),
            "bout": np.ascontiguousarray(bout),
            "ones": ones,
            "ident": ident,
        })
    return in_maps


_NC_CACHE = {}


def kernel(x, W_ih, W_hh, b_ih, b_hh, W_out, b_out, _t_steps=T):
    if _t_steps not in _NC_CACHE:
        _NC_CACHE[_t_steps] = _build(_t_steps)
    nc = _NC_CACHE[_t_steps]
    in_maps = _prep_inputs(x, W_ih, W_hh, b_ih, b_hh, W_out, b_out)
    res = run_bass_kernel_spmd(nc, in_maps, list(range(NC)))
    parts = [res.results[j]["outT"] for j in range(NC)]   # each [ES, T, B]
    full = np.concatenate(parts, axis=0)                  # [E, T, B]
    return np.ascontiguousarray(full.transpose(2, 1, 0))  # [B, T, E]
